# revision 65
# baseline (speedup 1.0000x reference)
"""Trainium2 Bass kernel for nn_Block_13391708030014 (dense transformer block).

Sharding: data-parallel over batch — core b computes batch item b entirely
(B == n_cores == 8), no collectives.

v2 structure (changes vs v1 in [brackets]):
  A. x loaded once into the persistent residual tile x2; ln1 token-major
     (bn_stats, one-group lookahead; [rsqrt via Ln+Exp so the scalar
     engine stays on the natural_log_exp table set all through phases
     A-C; normalize on scalar as Identity(x*rs - mean*rs)]); PE-transpose
     to feature-major hcT8 [fp8, kb-paired layout]; q via [fp8 DoubleRow]
     matmuls.
  B. spatial-reduction conv as [16 fp8-DoubleRow kb-paired] accumulated
     matmuls on strided views of hcT8; srn layernorm; k^T, kbd
     block-diagonal layout; v -> va = alpha*v (pos path) and
     vts = (1-alpha)*v (attn path).
  C. attention per 4-token-tile group, [emission ordered so the in-order
     PE queue never stalls: QK for BOTH head groups first (16 matmuls),
     then the dependency-free pos@va matmuls (col-tiled 4 heads
     concurrent), then the es-transposes/attn@v]. Exp+accum_out on
     [128,256] tiles gives unnormalized es (bf16) + row sums; recips
     batched [128,4]; [1/Z folded into a diag(recip) moving operand of
     the es-transpose matmuls (regular matmuls, fp32 PSUM -> one bf16
     eviction per [128,512])] and (1-alpha) pre-folded into vts.
     proj + residual in-place into x2; ln2 + h2T8 [fp8] folded in.
  D. [software-pipelined: fc1 runs two hidden-blocks ahead of the
     depthwise conv]. fc1 as [plain fp8 (FWL) over kb blocks] into a
     zero-padded 66x66 fp8 layout [two planes: original and +1-shifted;
     tap pairs chosen so two pairs live inside plane 0 with a 16-aligned
     +64 stride: (2,3),(5,6) at +64, (0,1),(7,8) across planes, center
     tap 4 single; plane-1 shift DMAs issued per 8-row chunk right
     behind fc1 evictions]. depthwise conv as fp8 DoubleRow diag-matmuls
     [tap-outer over row-block PAIRS with a double-buffered [P,2,512]
     PSUM tile so Gelu drains never stall the PE]; bias+Gelu fused into
     the PSUM eviction [emitted at low scheduling priority + virtual-time
     0.2ms so the scheduler doesn't interleave Gelu with phase C's tail
     Exp/Ln (activation-table thrash)]; fc2 fp8 DoubleRow as v1,
     residual added into x2, DMA'd out per 4-token-tile group.

Matmuls: fp8 (q, sr-conv, fc1, dw, fc2), bf16 (QK, pos, attn@v, proj,
k, v, transposes); residual stream fp32. HW-measured ~410 us/core,
rel err ~4.3e-3 (vs 654 us / 7e-4 for v1).
"""

from contextlib import ExitStack

import numpy as np
import ml_dtypes

import concourse.bass as bass
import concourse.tile as tile
from concourse import mybir
from concourse.ap import AP
from concourse.bass_utils import run_bass_kernel_spmd
from concourse.masks import make_identity

F32 = mybir.dt.float32
BF16 = mybir.dt.bfloat16
FP8 = mybir.dt.float8e4
AF = mybir.ActivationFunctionType
OP = mybir.AluOpType
DR = mybir.MatmulPerfMode.DoubleRow

B, N, C = 8, 4096, 256
H, DH = 8, 32
NK = 256
HID = 1024
HW = 64
SR = 4
P = 128
TT = N // P          # 32 token tiles
KB = C // P          # 2 channel blocks
MB = HID // P        # 8 hidden blocks
PADW = HW + 2        # 66
NPAD = PADW * PADW   # 4356
NPAD_AL = 4368       # NPAD padded to a 16-multiple


def _split_drain_waits(nc, max_waits=1):
    """walrus refuses instructions with more than one sem wait; hoist extra
    waits onto dedicated single-wait NOPs on the same engine."""
    for f in nc.m.functions:
        for blk in f.blocks:
            insts = blk.instructions
            new = []
            changed = False
            for inst in insts:
                si = getattr(inst, "sync_info", None)
                if si is not None and si.on_wait and len(si.on_wait) > max_waits:
                    for i, w in enumerate(list(si.on_wait)):
                        new.append(mybir.InstNoOp(
                            name=f"{inst.name}-ws{i}",
                            sync_info=mybir.SyncInfo(on_wait=[w], on_update=[]),
                            bass_nofuse=True,
                            engine=inst.engine,
                        ))
                    inst.sync_info = mybir.SyncInfo(
                        on_wait=[], on_update=list(si.on_update or []))
                    changed = True
                new.append(inst)
            if changed:
                blk.instructions = new


def _bf(x):
    return np.ascontiguousarray(x.astype(ml_dtypes.bfloat16))


def _f8(x):
    return np.ascontiguousarray(x.astype(ml_dtypes.float8_e4m3))


def _pair_kb(w):
    """[C, M] -> DoubleRow kb-paired stationary [Ki=128, Ko=2, M]."""
    ki, m = w.shape
    assert ki == C
    return np.ascontiguousarray(w.reshape(2, P, m).transpose(1, 0, 2))


def _prep_weights(i):
    """Fold LN affines into downstream weights; return DRAM payloads."""
    ln1_w, ln1_b = i["ln1_w"], i["ln1_b"]
    srn_w, srn_b = i["srn_w"], i["srn_b"]
    ln2_w, ln2_b = i["ln2_w"], i["ln2_b"]

    qw = ln1_w[:, None] * i["q_w"]                      # [C, C]
    qb = ln1_b @ i["q_w"] + i["q_b"]                    # [C]

    # sr_w is OIHW: [c_out, c_in, dy, dx] -> srw[tap, ci, co]
    srw = (i["sr_w"] * ln1_w[None, :, None, None]).transpose(2, 3, 1, 0)
    srw = np.ascontiguousarray(srw.reshape(SR * SR, C, C))
    srb = i["sr_b"] + np.einsum("i,oihw->o", ln1_b, i["sr_w"])
    # fp8 DR: [Ki=128, tap, Ko=2, C]
    srw8 = np.ascontiguousarray(
        srw.reshape(16, 2, P, C).transpose(2, 0, 1, 3))

    kvw = srn_w[:, None] * i["kv_w"]                    # [C, 2C]
    kvb = srn_b @ i["kv_w"] + i["kv_b"]
    kw, vw = kvw[:, :C], kvw[:, C:]
    kb_, vb = kvb[:C], kvb[C:]

    f1w = ln2_w[:, None] * i["fc1_w"]                   # [C, HID]
    f1b = ln2_b @ i["fc1_w"] + i["fc1_b"]

    # depthwise conv -> fp8 block-diag matrices. Pairs chosen so the
    # padded-layout pair stride is 16-aligned with only 2 planes:
    #   taps (2,3): plane0@2  / plane0@66   step  64
    #   taps (5,6): plane0@68 / plane0@132  step  64
    #   taps (0,1): plane0@0  / plane1@0    step  NPAD_AL
    #   taps (7,8): plane0@133/ plane1@133  step  NPAD_AL
    #   tap 4 (center) single.
    dww = i["dw_w"].reshape(HID, 9)                     # [HID, tap]
    idx = np.arange(P)
    dwdp = np.zeros((4, MB, P, 2, P), np.float32)
    dwds = np.zeros((MB, P, P), np.float32)
    for j, (ta, tb) in enumerate(((2, 3), (5, 6), (0, 1), (7, 8))):
        for mb in range(MB):
            dwdp[j, mb, idx, 0, idx] = dww[mb * P:(mb + 1) * P, ta]
            dwdp[j, mb, idx, 1, idx] = dww[mb * P:(mb + 1) * P, tb]
    for mb in range(MB):
        dwds[mb, idx, idx] = dww[mb * P:(mb + 1) * P, 4]

    # fc2 as fp8 DoubleRow over hidden-block pairs: [4, 128, 2, C]
    f2w8 = np.ascontiguousarray(
        i["fc2_w"].reshape(MB // 2, 2, P, C).transpose(0, 2, 1, 3))

    return {
        "qw8": _f8(_pair_kb(qw)), "qb": qb.astype(np.float32),
        "srw8": _f8(srw8), "srb": srb.astype(np.float32),
        "kw": _bf(kw), "kb": kb_.astype(np.float32),
        "vw": _bf(vw), "vb": vb.astype(np.float32),
        "pjw": _bf(i["proj_w"]), "pjb": i["proj_b"].astype(np.float32),
        "f1w8": _f8(_pair_kb(f1w)), "f1b": f1b.astype(np.float32),
        "dwdp": _f8(dwdp), "dwds": _f8(dwds),
        "dwb": i["dw_b"].astype(np.float32),
        "f2w8": _f8(f2w8), "f2b": i["fc2_b"].astype(np.float32),
    }


def _build_program(a, nz):
    nc = bass.Bass("TRN2", target_bir_lowering=False, debug=False,
                   num_devices=B)

    x_d = nc.dram_tensor("x", [N, C], F32, kind="ExternalInput").ap()
    # pos_2D, host-transposed per head to [NK, N] and cast to bf16
    post_d = nc.dram_tensor("post", [H, NK, N], BF16, kind="ExternalInput").ap()
    out_d = nc.dram_tensor("out", [N, C], F32, kind="ExternalOutput").ap()

    w_d = {}
    wshapes = {
        "qw8": ([P, 2, C], FP8), "srw8": ([P, 16, 2, C], FP8),
        "kw": ([C, C], BF16), "vw": ([C, C], BF16),
        "pjw": ([C, C], BF16), "f1w8": ([P, 2, HID], FP8),
        "dwdp": ([4, MB, P, 2, P], FP8), "dwds": ([MB, P, P], FP8),
        "f2w8": ([MB // 2, P, 2, C], FP8),
    }
    if nz["dwb"]:
        wshapes["dwb"] = ([HID], F32)
    for nm in ("qb", "srb", "kb", "vb", "pjb", "f1b", "f2b"):
        if nz[nm]:
            wshapes[nm] = ([{"f1b": HID}.get(nm, C)], F32)
    for nm, (shp, dt) in wshapes.items():
        w_d[nm] = nc.dram_tensor(nm, shp, dt, kind="ExternalInput").ap()

    scale = DH ** -0.5

    with tile.TileContext(nc) as tc, ExitStack() as ctx:
        persist = ctx.enter_context(tc.tile_pool(name="persist", bufs=1))
        wpool = ctx.enter_context(tc.tile_pool(name="weights", bufs=1))
        stat = ctx.enter_context(tc.tile_pool(name="stat", bufs=8))

        # ---- persistent tiles
        hcT8 = persist.tile([P, 2, N], FP8, tag="hcT8")
        qT = [persist.tile([P, N], BF16, tag=f"qT{k}", name=f"qT{k}")
              for k in range(KB)]
        kT = [persist.tile([P, NK], BF16, tag=f"kT{k}", name=f"kT{k}")
              for k in range(KB)]
        kbd = [persist.tile([P, 2, 512], BF16, tag=f"kbd{g}", name=f"kbd{g}")
               for g in range(KB)]
        vts = [persist.tile([P, C], BF16, tag=f"vts{k}", name=f"vts{k}")
               for k in range(KB)]
        va = [persist.tile([P, C], BF16, tag=f"va{k}", name=f"va{k}")
              for k in range(KB)]
        x2 = persist.tile([P, TT, C], F32, tag="x2")
        h2T8 = persist.tile([P, 2, N], FP8, tag="h2T8")

        # ---- x straight into the residual tile (loaded once); emitted
        # before the weight DMAs so phase A's stats start immediately
        xr = x_d.rearrange("(g q p) c -> g p q c", p=P, q=4)
        for g in range(TT // 4):
            nc.sync.dma_start(x2[:, g * 4:(g + 1) * 4, :], xr[g])

        # ---- constants / weights to SBUF
        ident = wpool.tile([P, P], BF16)
        make_identity(nc, ident[:])
        identf8 = wpool.tile([P, P], FP8)
        make_identity(nc, identf8[:])
        ident4 = wpool.tile([P, 4, P], BF16)
        for q in range(4):
            make_identity(nc, ident4[:, q, :])
        eps1 = wpool.tile([P, 1], F32)
        nc.vector.memset(eps1[:], 1e-6)
        epss = wpool.tile([P, 1], F32)
        nc.vector.memset(epss[:], 1e-5)

        qw_sb = wpool.tile([P, 2, C], FP8)
        nc.sync.dma_start(qw_sb[:], w_d["qw8"])
        srw_sb = wpool.tile([P, 16, 2, C], FP8)
        nc.sync.dma_start(srw_sb[:], w_d["srw8"])
        kw_sb = wpool.tile([P, KB, C], BF16)
        nc.sync.dma_start(kw_sb[:], w_d["kw"].rearrange("(k p) c -> p k c", p=P))
        vw_sb = wpool.tile([P, KB, C], BF16)
        nc.sync.dma_start(vw_sb[:], w_d["vw"].rearrange("(k p) c -> p k c", p=P))
        pjw_sb = wpool.tile([P, KB, C], BF16)
        nc.sync.dma_start(pjw_sb[:], w_d["pjw"].rearrange("(k p) c -> p k c", p=P))
        f1w_sb = wpool.tile([P, 2, HID], FP8)
        nc.sync.dma_start(f1w_sb[:], w_d["f1w8"])
        f2w_sb = wpool.tile([P, MB // 2, 2, C], FP8)
        nc.sync.dma_start(f2w_sb[:],
                          w_d["f2w8"].rearrange("g p two c -> p g two c"))
        if nz["dwb"]:
            dwb_sb = wpool.tile([P, MB], F32)
            nc.sync.dma_start(dwb_sb[:], w_d["dwb"].rearrange("(m p) -> p m", p=P))

        bias_sb = {}
        for nm, dim in (("qb", C), ("srb", C), ("kb", C), ("f1b", HID)):
            if nz[nm]:
                t = wpool.tile([P, dim // P], F32, name=f"bias_{nm}")
                nc.sync.dma_start(t[:], w_d[nm].rearrange("(k p) -> p k", p=P))
                bias_sb[nm] = t
        for nm in ("vb", "pjb", "f2b"):
            if nz[nm]:  # free-axis bias: broadcast across partitions
                t = wpool.tile([P, C], F32, name=f"biasbc_{nm}")
                nc.sync.dma_start(t[:], w_d[nm].to_broadcast([P, C]))
                bias_sb[nm] = t

        def rsqrt_batched(var_view, eps_tile, rs_out):
            """rs_out = (var+eps)^-0.5 via Ln+Exp (stays on the exp table
            set, unlike Sqrt). var_view/rs_out: [P, k]."""
            lnv = stat.tile(list(rs_out.shape), F32, tag="lnv", name="lnv")
            nc.scalar.activation(lnv[:], var_view, AF.Ln, bias=eps_tile[:])
            nc.scalar.activation(rs_out, lnv[:], AF.Exp, scale=-0.5)

        def ln_prep(mv, eps_tile, rs, nmb):
            """From bn_aggr output mv [P,k,2] compute rs = rsqrt(var+eps)
            and nmb = -mean*rs, so tokens normalize on the scalar engine as
            Identity(x*rs + nmb). rs/nmb: [P,k]."""
            rsqrt_batched(
                mv[:, :, 1:2].rearrange("p q one -> p (q one)"),
                eps_tile, rs)
            nc.vector.scalar_tensor_tensor(
                out=nmb, in0=mv[:, :, 0:1].rearrange("p q one -> p (q one)"),
                scalar=-1.0, in1=rs, op0=OP.mult, op1=OP.mult)

        # ========== phase A: ln1 + transpose + q =====================
        with ExitStack() as pctx:
            hcpool = pctx.enter_context(tc.tile_pool(name="hca", bufs=4))
            mvpool = pctx.enter_context(tc.tile_pool(name="mva", bufs=2))
            tpA = pctx.enter_context(
                tc.tile_pool(name="tpA", bufs=4, space="PSUM"))
            qa_ps = pctx.enter_context(
                tc.tile_pool(name="qaps", bufs=2, space="PSUM"))

            # one-group stats lookahead so the transpose/q stream of group g
            # never waits on its own group's stats chain
            def a_stats(g):
                mv = mvpool.tile([P, 4, 2], F32, name=f"mva{g % 2}")
                for t4 in range(4):
                    st = stat.tile([P, 6], F32, tag="stA", name="stA")
                    nc.vector.bn_stats(out=st[:], in_=x2[:, g * 4 + t4, :])
                    nc.vector.bn_aggr(out=mv[:, t4, :], in_=st[:])
                rs = stat.tile([P, 4], F32, tag="rsA", name="rsA")
                nmb = stat.tile([P, 4], F32, tag="nmA", name="nmA")
                ln_prep(mv, eps1, rs[:], nmb[:])
                return rs, nmb

            pending = a_stats(0)
            for g in range(TT // 4):
                rs, nmb = pending
                if g + 1 < TT // 4:
                    pending = a_stats(g + 1)
                for t4 in range(4):
                    tt = g * 4 + t4
                    hc = hcpool.tile([P, C], BF16, name="hc")
                    nc.scalar.activation(
                        hc[:], x2[:, tt, :], AF.Identity,
                        bias=nmb[:, t4:t4 + 1], scale=rs[:, t4:t4 + 1])
                    for kb in range(KB):
                        pt = tpA.tile([P, P], BF16, name="ptA")
                        nc.tensor.transpose(
                            pt[:], hc[:, kb * P:(kb + 1) * P], ident[:])
                        if (tt * 2 + kb) % 2:
                            nc.scalar.copy(
                                out=hcT8[:, kb, tt * P:(tt + 1) * P],
                                in_=pt[:])
                        else:
                            nc.vector.tensor_copy(
                                out=hcT8[:, kb, tt * P:(tt + 1) * P], in_=pt[:])
                # q for this 512-token group: fp8 DR over kb pairs
                nt = g
                for cb in range(KB):
                    ps = qa_ps.tile([P, 512], F32, name="qps")
                    nc.tensor.matmul(
                        ps[:], qw_sb[:, :, cb * P:(cb + 1) * P],
                        hcT8[:, :, nt * 512:(nt + 1) * 512],
                        start=True, stop=True, perf_mode=DR)
                    dst = qT[cb][:, nt * 512:(nt + 1) * 512]
                    if nz["qb"]:
                        nc.vector.tensor_scalar(
                            out=dst, in0=ps[:],
                            scalar1=bias_sb["qb"][:, cb:cb + 1],
                            scalar2=None, op0=OP.add)
                    else:
                        nc.vector.tensor_copy(out=dst, in_=ps[:])

        # ========== phase B: SR-conv, srn, k, v ======================
        with ExitStack() as pctx:
            mm_ps = pctx.enter_context(
                tc.tile_pool(name="mmB", bufs=3, space="PSUM"))
            tpB = pctx.enter_context(
                tc.tile_pool(name="tpB", bufs=4, space="PSUM"))
            bwork = pctx.enter_context(tc.tile_pool(name="bwork", bufs=1))

            # SR conv -> hsT (feature-major [co, nk]); fp8 DR over kb pairs
            hsT = [bwork.tile([P, NK], BF16, tag=f"hsT{c}", name=f"hsT{c}")
                   for c in range(KB)]
            conv_rhs = hcT8[:].rearrange(
                "p k (r a c b) -> p k a b r c", a=SR, b=SR, c=HW // SR)
            for cob in range(KB):
                ps = mm_ps.tile([P, NK], F32, tag="mm", name="psconv")
                for tap in range(16):
                    dy, dx = tap // SR, tap % SR
                    nc.tensor.matmul(
                        ps[:], srw_sb[:, tap, :, cob * P:(cob + 1) * P],
                        conv_rhs[:, :, dy, dx, :, :],
                        start=(tap == 0), stop=(tap == 15), perf_mode=DR)
                if nz["srb"]:
                    nc.vector.tensor_scalar(
                        out=hsT[cob][:], in0=ps[:],
                        scalar1=bias_sb["srb"][:, cob:cob + 1],
                        scalar2=None, op0=OP.add)
                else:
                    nc.vector.tensor_copy(out=hsT[cob][:], in_=ps[:])

            # srn layernorm (transpose -> stats -> normalize -> transpose)
            hs_tok = [bwork.tile([P, C], BF16, tag=f"hstok{k}",
                                 name=f"hstok{k}") for k in range(KB)]
            for nkb in range(KB):
                for cb in range(KB):
                    pt = tpB.tile([P, P], BF16, tag="ptB", name="ptB")
                    nc.tensor.transpose(
                        pt[:], hsT[cb][:, nkb * P:(nkb + 1) * P], ident[:])
                    nc.vector.tensor_copy(
                        out=hs_tok[nkb][:, cb * P:(cb + 1) * P], in_=pt[:])
            hsnT = [bwork.tile([P, NK], BF16, tag=f"hsnT{k}", name=f"hsnT{k}")
                    for k in range(KB)]
            for nkb in range(KB):
                st = stat.tile([P, 6], F32, tag="stB", name="stB")
                nc.vector.bn_stats(out=st[:], in_=hs_tok[nkb][:])
                mv = stat.tile([P, 2], F32, tag="mvB", name="mvB")
                nc.vector.bn_aggr(out=mv[:], in_=st[:])
                rs = stat.tile([P, 1], F32, tag="rsB", name="rsB")
                rsqrt_batched(mv[:, 1:2], epss, rs[:])
                hsn = bwork.tile([P, C], BF16, tag=f"hsn{nkb}",
                                 name=f"hsn{nkb}")
                nc.vector.tensor_scalar(
                    out=hsn[:], in0=hs_tok[nkb][:],
                    scalar1=mv[:, 0:1], scalar2=rs[:],
                    op0=OP.subtract, op1=OP.mult)
                for cb in range(KB):
                    pt = tpB.tile([P, P], BF16, tag="ptB", name="ptB2")
                    nc.tensor.transpose(
                        pt[:], hsn[:, cb * P:(cb + 1) * P], ident[:])
                    nc.vector.tensor_copy(
                        out=hsnT[cb][:, nkb * P:(nkb + 1) * P], in_=pt[:])

            # k^T [c, nk]
            for cb in range(KB):
                ps = mm_ps.tile([P, NK], F32, tag="mm", name="psk")
                for kb in range(KB):
                    nc.tensor.matmul(
                        ps[:], kw_sb[:, kb, cb * P:(cb + 1) * P], hsnT[kb][:],
                        start=(kb == 0), stop=(kb == KB - 1))
                if nz["kb"]:
                    nc.vector.tensor_scalar(
                        out=kT[cb][:], in0=ps[:],
                        scalar1=bias_sb["kb"][:, cb:cb + 1],
                        scalar2=None, op0=OP.add)
                else:
                    nc.vector.tensor_copy(out=kT[cb][:], in_=ps[:])
            # block-diag head-pair layout for batched QK
            for hg in range(KB):
                nc.vector.memset(kbd[hg][:], 0.0)
                for hh in range(4):
                    j, half = hh // 2, hh % 2
                    nc.vector.tensor_copy(
                        out=kbd[hg][hh * 32:(hh + 1) * 32, j,
                                    half * 256:(half + 1) * 256],
                        in_=kT[hg][hh * 32:(hh + 1) * 32, :])
            # v token-major [nk, c]; va = alpha*v for the pos path,
            # vts = (1-alpha)/NK * v for the (unnormalized-es)@v path
            for nkb in range(KB):
                ps = mm_ps.tile([P, C], F32, tag="mm", name="psv")
                for kb in range(KB):
                    nc.tensor.matmul(
                        ps[:], hsnT[kb][:, nkb * P:(nkb + 1) * P],
                        vw_sb[:, kb, :],
                        start=(kb == 0), stop=(kb == KB - 1))
                if nz["vb"]:
                    vt = bwork.tile([P, C], F32, tag=f"vtmp{nkb}",
                                    name=f"vtmp{nkb}")
                    nc.vector.tensor_add(
                        out=vt[:], in0=ps[:], in1=bias_sb["vb"][:])
                    src = vt[:]
                else:
                    src = ps[:]
                nc.vector.tensor_scalar(
                    out=va[nkb][:], in0=src, scalar1=a,
                    scalar2=None, op0=OP.mult)
                nc.vector.tensor_scalar(
                    out=vts[nkb][:], in0=src, scalar1=1.0 - a,
                    scalar2=None, op0=OP.mult)

        # ========== phase C: attention (+ ln2/h2T folded in) ==========
        with ExitStack() as pctx:
            pospool = pctx.enter_context(tc.tile_pool(name="pos", bufs=4))
            espool = pctx.enter_context(tc.tile_pool(name="eatt", bufs=3))
            etsbp = pctx.enter_context(tc.tile_pool(name="etsb", bufs=3))
            dpool = pctx.enter_context(tc.tile_pool(name="diag", bufs=10))
            otpool = pctx.enter_context(tc.tile_pool(name="otp", bufs=2))
            h2cpool = pctx.enter_context(tc.tile_pool(name="h2cc", bufs=3))
            mvpool = pctx.enter_context(tc.tile_pool(name="mvc", bufs=2))
            s_ps = pctx.enter_context(
                tc.tile_pool(name="sps", bufs=2, space="PSUM"))
            et_ps = pctx.enter_context(
                tc.tile_pool(name="etps", bufs=2, space="PSUM"))
            o_ps = pctx.enter_context(
                tc.tile_pool(name="ops", bufs=1, space="PSUM"))
            pj_ps = pctx.enter_context(
                tc.tile_pool(name="pjps", bufs=1, space="PSUM"))

            for ttg in range(8):
                # --- QK + Exp (with per-head row-sum accumulators) for
                # BOTH head groups first: the PE streams 16 independent QK
                # matmuls while the scalar engine chases with Exp, so the
                # PE never stalls waiting on the softmax chain.
                esl, D4l = [], []
                for hg in range(KB):
                    es = espool.tile([P, 16, NK + 16], BF16, name="es")
                    D4s = []
                    for t4 in range(4):
                        tt = ttg * 4 + t4
                        zs = stat.tile([P, 4], F32, tag="zs", name="zs")
                        for j in range(2):
                            sps = s_ps.tile([P, 512], F32, name="sps")
                            nc.tensor.matmul(
                                sps[:], qT[hg][:, tt * P:(tt + 1) * P],
                                kbd[hg][:, j, :], start=True, stop=True)
                            for half in range(2):
                                jh = j * 2 + half
                                nc.scalar.activation(
                                    es[:, t4 * 4 + jh, 0:NK],
                                    sps[:, half * 256:(half + 1) * 256],
                                    AF.Exp, scale=scale,
                                    accum_out=zs[:, jh:jh + 1])
                        rc4 = stat.tile([P, 4], F32, tag="rc4", name="rc4")
                        nc.vector.reciprocal(rc4[:], zs[:])
                        d4 = dpool.tile([P, 4, P], BF16, name="d4")
                        eng = (nc.vector, nc.gpsimd)[t4 % 2]
                        eng.tensor_tensor(
                            out=d4[:], in0=ident4[:],
                            in1=rc4[:].to_broadcast([P, 4, P]),
                            op=OP.mult)
                        D4s.append(d4)
                    esl.append(es)
                    D4l.append(D4s)
                # --- pos path for both head groups (independent of the
                # softmax chain -- fills the PE while Exp/diag complete)
                opts = []
                for hg in range(KB):
                    op_t = o_ps.tile([P, 512], F32, name=f"opt{hg}")
                    for nkb in range(KB):
                        pos_sb = pospool.tile([P, 4, 512], BF16, name="possb")
                        # keep the prefetch off the startup DMA burst (x2
                        # chunks + weights); pos isn't needed until ~75us
                        with tc.tile_wait_until(0.03):
                            nc.sync.dma_start(
                                pos_sb[:],
                                post_d.rearrange(
                                    "(g hh) nk n -> g nk hh n", g=KB)[
                                    hg, nkb * P:(nkb + 1) * P, :,
                                    ttg * 512:(ttg + 1) * 512])
                        for hh in range(4):
                            h = hg * 4 + hh
                            nc.tensor.matmul(
                                op_t[hh * 32:(hh + 1) * 32, :],
                                va[nkb][:, h * 32:(h + 1) * 32],
                                pos_sb[:, hh, :],
                                start=(nkb == 0), stop=False,
                                tile_position=(0, hh * 32))
                    opts.append(op_t)
                # --- es^T via diag-scaled regular matmuls, then @vts
                oTs = []
                for hg in range(KB):
                    es, D4s, op_t = esl[hg], D4l[hg], opts[hg]
                    for hh in range(4):
                        h = hg * 4 + hh
                        etsb = etsbp.tile([P, 2, 512], BF16, name="etsb")
                        for nkb in range(KB):
                            et = et_ps.tile([P, 512], F32, name="et")
                            for t4 in range(4):
                                nc.tensor.matmul(
                                    et[:, t4 * P:(t4 + 1) * P],
                                    es[:, t4 * 4 + hh,
                                       nkb * P:(nkb + 1) * P],
                                    D4s[t4][:, hh, :],
                                    start=True, stop=True)
                            nc.vector.tensor_copy(
                                out=etsb[:, nkb, :], in_=et[:])
                        for nkb in range(KB):
                            nc.tensor.matmul(
                                op_t[hh * 32:(hh + 1) * 32, :],
                                vts[nkb][:, h * 32:(h + 1) * 32],
                                etsb[:, nkb, :],
                                start=False, stop=(nkb == KB - 1),
                                tile_position=(0, hh * 32))
                    ot = otpool.tile([P, 512], BF16, tag=f"oTs{hg}",
                                     name=f"oTs{hg}")
                    nc.vector.tensor_copy(out=ot[:], in_=op_t[:])
                    oTs.append(ot)
                # proj + residual + ln2/h2T for the 4 token tiles
                mv = mvpool.tile([P, 4, 2], F32, name="mvc")
                for t4 in range(4):
                    tt = ttg * 4 + t4
                    pps = pj_ps.tile([P, C], F32, tag="pps", name="pps",
                                     bufs=1)
                    for hg in range(KB):
                        nc.tensor.matmul(
                            pps[:], oTs[hg][:, t4 * P:(t4 + 1) * P],
                            pjw_sb[:, hg, :],
                            start=(hg == 0), stop=(hg == KB - 1))
                    if nz["pjb"]:
                        nc.vector.tensor_tensor(
                            out=x2[:, tt, :], in0=x2[:, tt, :], in1=pps[:],
                            op=OP.add)
                        nc.vector.tensor_add(
                            out=x2[:, tt, :], in0=x2[:, tt, :],
                            in1=bias_sb["pjb"][:])
                    else:
                        nc.vector.tensor_tensor(
                            out=x2[:, tt, :], in0=x2[:, tt, :], in1=pps[:],
                            op=OP.add)
                    st = stat.tile([P, 6], F32, tag="stC", name="stC")
                    nc.vector.bn_stats(out=st[:], in_=x2[:, tt, :])
                    nc.vector.bn_aggr(out=mv[:, t4, :], in_=st[:])
                rs = stat.tile([P, 4], F32, tag="rsC", name="rsC")
                nmb = stat.tile([P, 4], F32, tag="nmC", name="nmC")
                ln_prep(mv, eps1, rs[:], nmb[:])
                for t4 in range(4):
                    tt = ttg * 4 + t4
                    h2c = h2cpool.tile([P, C], BF16, name="h2c")
                    nc.vector.tensor_scalar(
                        out=h2c[:], in0=x2[:, tt, :],
                        scalar1=rs[:, t4:t4 + 1], scalar2=nmb[:, t4:t4 + 1],
                        op0=OP.mult, op1=OP.add)
                    for kb in range(KB):
                        pt = pj_ps.tile([P, P], BF16, tag="tpC", name="ptC",
                                        bufs=1)
                        nc.tensor.transpose(
                            pt[:], h2c[:, kb * P:(kb + 1) * P], ident[:])
                        # last ttg: keep the scalar engine out of the h2T
                        # path -- its table-switch-laden tail would gate
                        # phase D's first dw matmuls
                        if kb and ttg < 7:
                            nc.scalar.copy(
                                out=h2T8[:, kb, tt * P:(tt + 1) * P],
                                in_=pt[:])
                        else:
                            nc.vector.tensor_copy(
                                out=h2T8[:, kb, tt * P:(tt + 1) * P],
                                in_=pt[:])

        # ========== phase D: MLP =====================================
        with ExitStack() as pctx:
            mpadp = pctx.enter_context(tc.tile_pool(name="mpad", bufs=3))
            m2cp = pctx.enter_context(tc.tile_pool(name="m2c", bufs=2))
            dwdp_p = pctx.enter_context(tc.tile_pool(name="dwd", bufs=3))
            f1_ps = pctx.enter_context(
                tc.tile_pool(name="f1ps", bufs=2, space="PSUM"))
            dw_ps = pctx.enter_context(
                tc.tile_pool(name="dwps", bufs=2, space="PSUM"))
            f2_ps = pctx.enter_context(
                tc.tile_pool(name="f2ps", bufs=2, space="PSUM"))

            outr = out_d.rearrange("(g q p) c -> g p q c", p=P, q=4)
            gelu_bias = 0.0
            stage = {}

            def fc1_stage(mb):
                """fc1 into mpad plane 0, per-chunk +1-shift into plane 1,
                and the dw weight DMAs for this mb."""
                mpad = mpadp.tile([P, 2, NPAD_AL], FP8, tag="mpad",
                                  name=f"mpad{mb}")
                vp = mpad[:, 0, 0:NPAD].rearrange("p (r c) -> p r c", c=PADW)
                nc.gpsimd.memset(vp[:, 0, :], 0.0)
                nc.gpsimd.memset(vp[:, PADW - 1, :], 0.0)
                nc.gpsimd.memset(vp[:, 1:PADW - 1, 0:1], 0.0)
                nc.gpsimd.memset(vp[:, 1:PADW - 1, PADW - 1:PADW], 0.0)
                for nt in range(8):
                    ps = f1_ps.tile([P, 512], F32, name="psf1")
                    # plain fp8 (not DoubleRow): FWL hides the weight loads,
                    # where DR's 256-col LDWEIGHTS was exposed
                    for kb in range(KB):
                        nc.tensor.matmul(
                            ps[:], f1w_sb[:, kb, mb * P:(mb + 1) * P],
                            h2T8[:, kb, nt * 512:(nt + 1) * 512],
                            start=(kb == 0), stop=(kb == KB - 1))
                    dst = vp[:, 1 + 8 * nt:1 + 8 * nt + 8, 1:65]
                    src = ps.rearrange("p (r c) -> p r c", c=HW)
                    if nz["f1b"]:
                        if nt % 2 == 0:
                            nc.scalar.activation(
                                dst, src, AF.Identity,
                                bias=bias_sb["f1b"][:, mb:mb + 1])
                        else:
                            nc.vector.tensor_scalar(
                                out=dst, in0=src,
                                scalar1=bias_sb["f1b"][:, mb:mb + 1],
                                scalar2=None, op0=OP.add)
                    elif nt % 2:
                        nc.vector.tensor_copy(out=dst, in_=src)
                    else:
                        nc.scalar.copy(out=dst, in_=src)
                    # plane 1 (+1 shift) for the 8 plane-0 rows finalized by
                    # this chunk (row 0 is the memset top border)
                    s0 = PADW * 8 * nt
                    e0 = PADW * 8 * (nt + 1) if nt < 7 else NPAD - 1
                    nc.sync.dma_start(
                        out=mpad[:, 1, s0:e0], in_=mpad[:, 0, s0 + 1:e0 + 1])
                dwp_sb = dwdp_p.tile([P, 4, 2, P], FP8, tag="dwdp",
                                     name=f"dwp{mb}")
                nc.sync.dma_start(
                    dwp_sb[:],
                    w_d["dwdp"][:, mb].rearrange("j q two c -> q j two c"))
                dws_sb = dwdp_p.tile([P, P], FP8, tag="dwds", name=f"dws{mb}")
                nc.sync.dma_start(
                    dws_sb[:], w_d["dwds"][mb].rearrange("q c -> q c"))
                if mb % 2 == 0:
                    stage[f"m2pair{mb // 2}"] = m2cp.tile(
                        [P, 2, N], FP8, tag="m2c", name=f"m2pair{mb}")
                stage[mb] = (mpad, dwp_sb, dws_sb)

            def dw_stage(mb):
                """depthwise conv: 4 fp8 DoubleRow pairs + 1 single,
                tap-outer over 4-row-block groups to amortize LDWEIGHTS."""
                mpad, dwp_sb, dws_sb = stage.pop(mb)
                mp_flat = mpad[:].rearrange("p q n -> p (q n)")
                pstride = mp_flat.ap[0][0]
                m2c = stage[f"m2pair{mb // 2}"][:, mb % 2, :]
                pair_off = (2, 68, 0, 133)
                pair_step = (64, 64, NPAD_AL, NPAD_AL)
                for rbg in (0, 2, 4, 6):
                    dps2 = dw_ps.tile([P, 2, 512], F32, tag="dps", name="dps")
                    for pj in range(4):
                        for rb2 in range(2):
                            rb = rbg + rb2
                            rhs = AP(
                                mp_flat.tensor,
                                mp_flat.offset + pair_off[pj] + PADW * 8 * rb,
                                [[pstride, P], [pair_step[pj], 2],
                                 [PADW, 8], [1, HW]])
                            nc.tensor.matmul(
                                dps2[:, rb2, :], dwp_sb[:, pj, :, :], rhs,
                                start=(pj == 0), stop=False, perf_mode=DR)
                    for rb2 in range(2):
                        rb = rbg + rb2
                        rhs = AP(
                            mp_flat.tensor,
                            mp_flat.offset + 67 + PADW * 8 * rb,
                            [[pstride, P], [PADW, 8], [1, HW]])
                        nc.tensor.matmul(
                            dps2[:, rb2, :], dws_sb[:], rhs,
                            start=False, stop=True)
                    # Emit Gelu at very low scheduling priority so the
                    # scheduler runs it as late as dependencies allow --
                    # otherwise it interleaves Gelu into phase C's tail
                    # Exp/Ln ops and the activation table thrashes
                    # (~2.7us per switch, stalling the PE behind it).
                    with tc.high_priority(offset=-1000000), \
                            tc.tile_wait_until(0.2):
                        for rb2 in range(2):
                            rb = rbg + rb2
                            bias = (dwb_sb[:, mb:mb + 1] if nz["dwb"]
                                    else gelu_bias)
                            nc.scalar.activation(
                                m2c[:, rb * 512:(rb + 1) * 512],
                                dps2[:, rb2, :], AF.Gelu, bias=bias)

            def fc2_stage(mbq):
                """fc2: fp8 DoubleRow over hidden-block pairs; final residual
                add into x2 (+ output DMA on the last group)."""
                pairs = [stage[f"m2pair{mbq // 2}"],
                         stage[f"m2pair{mbq // 2 + 1}"]]
                for tt in range(TT):
                    fps = f2_ps.tile([P, C], F32, name="fps")
                    for j in range(2):
                        nc.tensor.matmul(
                            fps[:], pairs[j][:, :, tt * P:(tt + 1) * P],
                            f2w_sb[:, mbq // 2 + j, :, :],
                            start=(j == 0), stop=(j == 1), perf_mode=DR)
                    nc.vector.tensor_tensor(
                        out=x2[:, tt, :], in0=x2[:, tt, :], in1=fps[:],
                        op=OP.add)
                    if mbq == 4:
                        if nz["f2b"]:
                            nc.vector.tensor_add(
                                out=x2[:, tt, :], in0=x2[:, tt, :],
                                in1=bias_sb["f2b"][:])
                        if tt % 4 == 3:
                            g = tt // 4
                            nc.sync.dma_start(
                                outr[g], x2[:, g * 4:(g + 1) * 4, :])

            # software pipeline: keep two fc1 stages in flight ahead of each
            # dw so the PE never stalls at a dw matmul waiting for fc1
            # evictions / plane shifts
            fc1_stage(0)
            fc1_stage(1)
            for mb in range(MB):
                if mb + 2 < MB:
                    fc1_stage(mb + 2)
                dw_stage(mb)
                if mb == 3:
                    fc2_stage(0)
            fc2_stage(4)

    _split_drain_waits(nc)
    return nc


def _run(inputs, trace=False):
    w = _prep_weights(inputs)
    a = float(np.asarray(inputs["alpha"]).reshape(-1)[0])
    nz = {nm: bool(np.any(w[nm])) for nm in
          ("qb", "srb", "kb", "vb", "pjb", "f1b", "f2b", "dwb")}
    nc = _build_program(a, nz)

    x = np.asarray(inputs["x"], np.float32)
    pos = np.asarray(inputs["pos_2D"], np.float32)
    shared = {k: v for k, v in w.items()
              if k in ("qw8", "srw8", "kw", "vw", "pjw", "f1w8", "dwdp",
                       "dwds", "f2w8")}
    for nm in ("qb", "srb", "kb", "vb", "pjb", "f1b", "f2b", "dwb"):
        if nz[nm]:
            shared[nm] = w[nm]
    in_maps = []
    for b in range(B):
        posT = np.ascontiguousarray(
            pos[b].transpose(0, 2, 1)).astype(ml_dtypes.bfloat16)
        in_maps.append(dict(shared, x=np.ascontiguousarray(x[b]), post=posT))
    res = run_bass_kernel_spmd(nc, in_maps, list(range(B)), trace=trace)
    out = np.stack([res.results[b]["out"] for b in range(B)]).astype(np.float32)
    return out, res


def kernel(**inputs) -> np.ndarray:
    out, _ = _run(inputs, trace=False)
    return out
